# revision 49
# baseline (speedup 1.0000x reference)
"""MGE velocity kernel for 8 Trainium2 NeuronCores.

out[n] = R_sc[n] * sqrt(vc2_mge(R2[n]) + vc2_bh(R2[n]))

Key observation: with the staged parameters (m_bh = 8.0), the black-hole
term vc2_bh = C0*R2^-1.5 dominates vc2_mge by >= 4 orders of magnitude over
the entire sampled R2 range [4.3e-4, 771]; dropping vc2_mge entirely gives
max rel err  max(1 - 1/sqrt(1 + mge/bh)) = 3.04e-5  (exact host-side bound,
recomputed per call from the runtime parameter vectors using the reference's
own 128-node quadrature).  The kernel then collapses to the power law

    v = e^k * r2u^-0.25,   k = 0.5*ln(G*10^m_bh*scale^2) - ln(scale)
      = Exp(-0.25 * Ln((alpha/B^2) * B^2*r2u)),   alpha = exp(-4k)

Fast-path device schedule (data parallel, 131072 points/core, [128,1024],
column-chunked [512,256,256] so later chunks' DMA/squares overlap earlier
chunks' ACT/stores; ~11.7us/core vs 151.7us for the Gaussian-sum kernel):
  - x,y,z arrive packed in one fp16 tensor per chunk (halves DMA bytes and
    HWDGE dispatch slots), pre-scaled by a power of two B so fp16 squares
    stay out of the denormal zone without overflowing
  - squares on DVE in fp16 (2x perf mode); GPSIMD takes z^2 of later
    chunks in parallel; TensorE sums the three squares into PSUM via
    identity matmuls (fp16, 1 cycle/row), kept at full p-state by no-dep
    warm-up matmuls; per-chunk PSUM tiles so accumulation groups don't
    serialize across chunks
  - ACT: Ln(scale*psum) with scale=alpha/B^2 as a [P,1] operand, then
    Exp(-0.25*x) -> v; Ln+Exp+Square live in one activation table set
    (enforced at compile time) loaded once at t=0 under the input DMAs
  - the alpha/W params ride the software-DGE path (Pool engine) keeping
    the shared HWDGE dispatcher free for input/output chunks
If the host-side bound says vc2_mge matters (different runtime params) or
the fp16 flush-to-zero risk check fails, falls back to the previous full
128-term Gaussian-sum kernel (unchanged, rel err 1.7e-5, 151.7us).
"""

import numpy as np
from numpy.polynomial.legendre import leggauss

N_CORES = 8
H = W = 1024
N = H * W
N_C = N // N_CORES        # 131072 points per core
P = 128
FN = N_C // P             # 1024 natural free dim
G_CONST = 0.004301
SOFT = 0.0

# fast path tuning (settled by TimelineSim sweeps)
CHS = [512, 256, 256]     # column chunk sizes (sum = FN)
ACT_ORDER = ["L0", "E0", "L1", "E1", "L2", "E2"]  # Ln/Exp issue order (hint)
LAST_OUT_ACT = False      # issue final store from the ACT queue
PE_FILL = 0               # extra junk warm-up matmuls (base warm-up always on)
GP_Z = (1, 2)             # chunks whose z^2 runs on GPSIMD (parallel to DVE)
GP_Z0A = False            # GPSIMD also takes z^2 of chunk-1's first half
OUT16 = True              # store v in fp16 (host upcasts; halves out DMA)
IN1_HALVES = True         # chunk-1 input/squares/mms as two column halves
IN1_SPLIT = 0             # asymmetric split column for chunk 1 (0 = half)

# general path (fallback) constants
G = 32
D = 4
F = N_C // G
QUAD = 8
K = 16
M = QUAD * K
NI = M // D

_BASS_CACHE = {}


class _single_act_table:
    """During compile, restrict the activation-table list to the one set
    that holds ln+exp+square together (index 6, natural_log_exp_and_others)
    so Bacc's table-load pass emits a single LoadActFuncSet instead of
    reloading on every Ln<->Exp transition. Positions of all 24 sets are
    preserved (other sets are emptied, not removed) so the emitted
    act_func_set_id still indexes act_info.json correctly."""

    def __enter__(self):
        from concourse import bacc
        self._orig = bacc.get_activation_tables

        def patched(arch):
            import concourse.mybir as mybir
            AF = mybir.ActivationFunctionType
            tabs = self._orig(arch)
            out = type(tabs)()
            for name, funcs in tabs.items():
                keep = AF.Ln in funcs and AF.Exp in funcs
                out[name] = funcs if keep else type(funcs)()
            return out

        bacc.get_activation_tables = patched
        return self

    def __exit__(self, *exc):
        from concourse import bacc
        bacc.get_activation_tables = self._orig
        return False


def _build_fast():
    if "fast" in _BASS_CACHE:
        return _BASS_CACHE["fast"]
    import concourse.mybir as mybir
    from concourse import bacc
    from concourse.tile import TileContext

    fp32 = mybir.dt.float32
    fp16 = mybir.dt.float16
    f32r = mybir.dt.float32r
    AF = mybir.ActivationFunctionType
    OP = mybir.AluOpType

    nc = bacc.Bacc("TRN2")
    # x, y, z packed and pre-scaled by B host-side: one DMA per column
    # chunk moves all three components
    xyz = nc.dram_tensor("xyz", [P, 3, FN], fp16, kind="ExternalInput")
    w_in = nc.dram_tensor("w_id", [P, P], fp16, kind="ExternalInput")
    # eplg[:,0] = alpha/B^2, alpha = exp(-4k): Ln(scale*psum) = ln(alpha*r2),
    # so the final Exp needs no operand: exp(-0.25*ln(alpha*r2)) = e^k*r2^-.25
    ep_in = nc.dram_tensor("eplg", [P, 1], fp32, kind="ExternalInput")
    out_dt = fp16 if OUT16 else fp32
    out = nc.dram_tensor("out", [P, FN], out_dt, kind="ExternalOutput")

    with TileContext(nc) as tc:
        with (
            tc.tile_pool(name="s", bufs=1) as sp,
            tc.tile_pool(name="ps", bufs=1, space="PSUM") as pp,
        ):
            # params ride the software-DGE path first (Pool engine is idle):
            # input chunks get the hardware dispatch queue to themselves
            w_t = sp.tile([P, P], fp16)
            ep_t = sp.tile([P, 1], fp32)
            nc.gpsimd.dma_start(w_t[:], w_in[:])
            nc.gpsimd.dma_start(ep_t[:], ep_in[:])

            # warm the ln/exp activation table while input DMAs stream
            dum = sp.tile([P, 1], fp32)
            dum2 = sp.tile([P, 1], fp32)
            nc.gpsimd.memset(dum[:], 1.0)
            nc.scalar.activation(dum2[:], dum[:], AF.Ln)
            nc.scalar.activation(dum[:], dum2[:], AF.Exp)

            # PE warm-up + fillers: junk matmuls with no deps run back-to-back
            # from t~0, keeping the p-state ramp hot until real work arrives
            jw = sp.tile([P, 1], fp16)
            jm = sp.tile([P, 256], fp16)
            jp = pp.tile([1, 256], fp32)
            nc.gpsimd.memset(jw[:], 0.0)
            nc.gpsimd.memset(jm[:], 0.0)
            for _ in range(1 + PE_FILL):
                nc.tensor.matmul(jp[:], jw[:], jm[:], start=True, stop=True)

            pk = sp.tile([P, 3, FN], fp16)
            sx = sp.tile([P, FN], fp16)
            sy = sp.tile([P, FN], fp16)
            sz = sp.tile([P, FN], fp16)

            edges = np.concatenate([[0], np.cumsum(CHS)])
            assert edges[-1] == FN
            sl = [slice(int(edges[c]), int(edges[c + 1])) for c in range(len(CHS))]
            # per-chunk PSUM/SBUF dests: accumulation groups on a shared
            # PSUM tile would serialize chunk c's Ln behind chunk c+1's mms
            ps_c = [pp.tile([P, w], fp32, name=f"ps{c}") for c, w in enumerate(CHS)]
            # ln intermediates in PSUM: ACT's PSUM access latency (172cyc)
            # beats SBUF (222cyc), shaving init time off every Ln/Exp
            l_c = [pp.tile([P, w], fp32, name=f"l{c}") for c, w in enumerate(CHS)]
            v_c = [sp.tile([P, w], out_dt, name=f"v{c}") for c, w in enumerate(CHS)]

            # sub-slices: chunk 1 optionally lands as two half-DMAs so its
            # first squares start ~550ns earlier (head is latency-critical)

            subs = []
            nsplit = int(IN1_HALVES)   # halve the first `nsplit` chunks
            for c in range(len(CHS)):
                if c < nsplit:
                    h = (IN1_SPLIT if (IN1_SPLIT and c == 0)
                         else CHS[c] // 2)
                    subs.append([(c, slice(0, h)), (c, slice(h, CHS[c]))])
                else:
                    subs.append([(c, slice(0, CHS[c]))])

            for c in range(len(CHS)):
                for cc, r in subs[c]:
                    s = slice(int(edges[cc]) + r.start, int(edges[cc]) + r.stop)
                    nc.sync.dma_start(pk[:, :, s], xyz[:, :, s])

            # squares on DVE (fp16 in/out hits the 2x perf mode); GPSIMD
            # takes z^2 of later chunks in parallel with the DVE stream
            if GP_Z0A and len(subs[0]) == 2:
                r = subs[0][0][1]
                s = slice(r.start, r.stop)
                nc.gpsimd.tensor_tensor(sz[:, s], pk[:, 2, s], pk[:, 2, s], OP.mult)
            for c in range(len(CHS)):
                if c in GP_Z:
                    s = sl[c]
                    nc.gpsimd.tensor_tensor(
                        sz[:, s], pk[:, 2, s], pk[:, 2, s], OP.mult
                    )
            for c in range(len(CHS)):
                for ci, (cc, r) in enumerate(subs[c]):
                    s = slice(int(edges[cc]) + r.start, int(edges[cc]) + r.stop)
                    nc.vector.tensor_tensor(sx[:, s], pk[:, 0, s], pk[:, 0, s], OP.mult)
                    nc.vector.tensor_tensor(sy[:, s], pk[:, 1, s], pk[:, 1, s], OP.mult)
                    if c not in GP_Z and not (GP_Z0A and c == 0 and ci == 0):
                        nc.vector.tensor_tensor(sz[:, s], pk[:, 2, s], pk[:, 2, s], OP.mult)
            for c in range(len(CHS)):
                # r2 = x^2+y^2+z^2 summed on the otherwise idle TensorE,
                # one matmul per 512-col PSUM bank, in square-completion order
                for cc, r in subs[c]:
                    for q, sq in ((0, sx), (1, sy), (2, sz)):
                        for b0 in range(r.start, r.stop, 512):
                            b1 = min(b0 + 512 - b0 % 512, r.stop)
                            nc.tensor.matmul(
                                ps_c[c][:, b0:b1], w_t[:],
                                sq[:, int(edges[c]) + b0 : int(edges[c]) + b1],
                                start=(q == 0), stop=(q == 2),
                            )
            last_exp = max(i for i, t in enumerate(ACT_ORDER) if t[0] == "E")
            for i, tok in enumerate(ACT_ORDER):
                c = int(tok[1:])
                if tok[0] == "L":
                    nc.scalar.activation(
                        l_c[c][:], ps_c[c][:], AF.Ln, scale=ep_t[:, 0:1]
                    )
                else:
                    nc.scalar.activation(v_c[c][:], l_c[c][:], AF.Exp, scale=-0.25)
                    if i == last_exp and LAST_OUT_ACT:
                        # final store issued by ACT itself: no cross-engine sem
                        nc.scalar.dma_start(out[:, sl[c]], v_c[c][:])
                    else:
                        nc.sync.dma_start(out[:, sl[c]], v_c[c][:])

    with _single_act_table():
        nc.compile()
    _BASS_CACHE["fast"] = nc
    return nc


def _build_bass():
    """Module used for the staged inputs (fast path); kept under the old
    name so external tracing harnesses pick up the kernel actually run."""
    return _build_fast()


def _quad_terms_f64(surf, sigma, qobs, M_to_L, inc, quad):
    """Host f64 reduction of the parameter vectors to per-term (b_m, c_m)
    for vc2_mge(r2u) = sum_m c_m * exp(-b_m * r2u). Mirrors reference.py."""
    surf = surf.astype(np.float64)
    sigma = sigma.astype(np.float64)
    qobs = qobs.astype(np.float64)
    cos_i, sin_i = np.cos(inc), np.sin(inc)
    q_intr = np.sqrt(qobs**2 - cos_i**2) / sin_i
    md = surf * M_to_L * qobs / (q_intr * sigma * np.sqrt(2.0 * np.pi))
    scale = np.quantile(sigma, 0.5)
    sig_sc = sigma / scale
    mds = np.quantile(sig_sc, 0.5)
    mxs = sig_sc.max()
    t_lo = np.arcsinh(np.log(1e-7 * mds) * 2.0 / np.pi)
    t_hi = np.arcsinh(np.log(1000.0 * mxs) * 2.0 / np.pi)
    xl, wl = leggauss(quad)
    t = 0.5 * (t_hi - t_lo) * xl + 0.5 * (t_hi + t_lo)
    w = 0.5 * (t_hi - t_lo) * wl
    u = np.exp(np.pi / 2.0 * np.sinh(t))
    du = np.pi / 2.0 * np.cosh(t) * u
    coef = q_intr * md
    inv_s2 = 1.0 / sig_sc**2
    b = ((0.5 / (1.0 + u))[:, None] * inv_s2[None, :]).ravel() / scale**2
    c = (
        (coef[None, :] / ((1.0 + u[:, None]) ** 2
                          * np.sqrt(q_intr[None, :] ** 2 + u[:, None])))
        * (du * w)[:, None]
    ).ravel() * (2.0 * np.pi * G_CONST * scale**2)
    return b, c, scale


def _fast_path_bound(x, y, z, surf, sigma, qobs, M_to_L, inc, m_bh):
    """Exact max-rel-err bound on v from dropping vc2_mge, over the actual
    sample r2 range, using the reference's own 128-node quadrature."""
    try:
        b, c, scale = _quad_terms_f64(surf, sigma, qobs, M_to_L, inc, 128)
        if not (np.all(np.isfinite(b)) and np.all(np.isfinite(c))):
            return np.inf, None
        x64 = x.astype(np.float64)
        r2 = x64 * x64
        y64 = y.astype(np.float64)
        r2 += y64 * y64
        z64 = z.astype(np.float64)
        r2 += z64 * z64
        r2min, r2max = float(r2.min()), float(r2.max())
        if not (np.isfinite(r2min) and np.isfinite(r2max)) or r2min <= 0:
            return np.inf, None
        C0 = G_CONST * 10.0 ** float(m_bh) * scale**2
        g = np.geomspace(r2min, r2max, 257)
        mge = np.exp(-np.outer(g, b)) @ c
        bh = C0 * g**-1.5
        bound = float(np.max(1.0 - 1.0 / np.sqrt(1.0 + mge / bh)))
        k = 0.5 * np.log(C0) - np.log(scale)
        if not np.isfinite(bound) or not np.isfinite(k):
            return np.inf, None
        return bound, float(k)
    except Exception:
        return np.inf, None


def _run_fast(x, y, z, k, B):
    from concourse.bass_utils import run_bass_kernel_spmd

    alpha = np.exp(-4.0 * k)
    w_id = np.eye(P, dtype=np.float16)
    eplg = np.full((P, 1), alpha / B**2, np.float32)
    xyz = np.empty((N_CORES, P, 3, FN), np.float16)
    xyz[:, :, 0, :] = (x * B).ravel().reshape(N_CORES, P, FN)
    xyz[:, :, 1, :] = (y * B).ravel().reshape(N_CORES, P, FN)
    xyz[:, :, 2, :] = (z * B).ravel().reshape(N_CORES, P, FN)
    in_maps = [{"xyz": xyz[i], "w_id": w_id, "eplg": eplg}
               for i in range(N_CORES)]
    nc = _build_fast()
    try:
        res = run_bass_kernel_spmd(nc, in_maps, core_ids=list(range(N_CORES)))
    except Exception:
        # transient NRT device hiccups have been observed on first contact
        res = run_bass_kernel_spmd(nc, in_maps, core_ids=list(range(N_CORES)))
    outs = [res.results[i]["out"].reshape(-1).astype(np.float32)
            for i in range(N_CORES)]
    return np.concatenate(outs).reshape(H, W)


# ---------------------------------------------------------------------------
# general fallback: full 128-term Gaussian-sum kernel (previous baseline)
# ---------------------------------------------------------------------------

def _build_general():
    if "general" in _BASS_CACHE:
        return _BASS_CACHE["general"]
    import concourse.mybir as mybir
    from concourse import bacc
    from concourse.tile import TileContext

    fp32 = mybir.dt.float32
    fp16 = mybir.dt.float16
    AF = mybir.ActivationFunctionType
    OP = mybir.AluOpType

    nc = bacc.Bacc("TRN2")
    xs = nc.dram_tensor("xs", [P, FN], fp32, kind="ExternalInput")
    ys = nc.dram_tensor("ys", [P, FN], fp32, kind="ExternalInput")
    zs = nc.dram_tensor("zs", [P, FN], fp32, kind="ExternalInput")
    w_in = nc.dram_tensor("w_red", [P, G], fp16, kind="ExternalInput")
    sc_in = nc.dram_tensor("scale_sb", [P, NI], fp32, kind="ExternalInput")
    bi_in = nc.dram_tensor("bias_sb", [P, NI], fp32, kind="ExternalInput")
    ep_in = nc.dram_tensor("eplg", [P, 4], fp32, kind="ExternalInput")
    out = nc.dram_tensor("out", [P, FN], fp32, kind="ExternalOutput")

    with TileContext(nc) as tc:
        with (
            tc.tile_pool(name="singles", bufs=1) as singles,
            tc.tile_pool(name="epool", bufs=4) as epool,
            tc.tile_pool(name="psum", bufs=1, space="PSUM") as psum,
        ):
            x_t = singles.tile([P, FN], fp32)
            y_t = singles.tile([P, FN], fp32)
            z_t = singles.tile([P, FN], fp32)
            w_t = singles.tile([P, G], fp16)
            sc_t = singles.tile([P, NI], fp32)
            bi_t = singles.tile([P, NI], fp32)
            ep_t = singles.tile([P, 4], fp32)
            nc.sync.dma_start(x_t[:], xs[:])
            nc.sync.dma_start(y_t[:], ys[:])
            nc.sync.dma_start(z_t[:], zs[:])
            nc.sync.dma_start(w_t[:], w_in[:])
            nc.sync.dma_start(sc_t[:], sc_in[:])
            nc.sync.dma_start(bi_t[:], bi_in[:])
            nc.sync.dma_start(ep_t[:], ep_in[:])

            r2 = singles.tile([P, FN], fp32)
            t2 = singles.tile([P, FN], fp32)
            sx = singles.tile([P, FN], fp32)
            nc.scalar.activation(sx[:], x_t[:], AF.Square)
            nc.vector.tensor_tensor(t2[:], y_t[:], y_t[:], OP.mult)
            nc.vector.tensor_tensor(r2[:], z_t[:], z_t[:], OP.mult)
            nc.vector.tensor_tensor(t2[:], t2[:], sx[:], OP.add)
            nc.vector.tensor_tensor(r2[:], r2[:], t2[:], OP.add)

            r2d = singles.tile([P, F], fp32)
            for j in range(D):
                for c in range(D):
                    nc.sync.dma_start(
                        r2d[G * j : G * (j + 1), FN * c : FN * (c + 1)],
                        r2[G * c : G * (c + 1), :],
                    )

            lnr2n = singles.tile([P, FN], fp32)
            nc.scalar.activation(lnr2n[:], r2[:], AF.Ln)
            bh_n = singles.tile([P, FN], fp32)
            nc.scalar.activation(
                bh_n[:], lnr2n[:], AF.Exp, bias=ep_t[:, 0:1], scale=-1.5
            )

            integ = psum.tile([G, F], fp32)
            for i in range(NI):
                e = epool.tile([P, F], fp16, tag="e")
                nch = D if i in (0, NI - 1) else 1
                cw = F // nch
                for ch in range(nch):
                    nc.scalar.activation(
                        e[:, cw * ch : cw * (ch + 1)],
                        r2d[:, cw * ch : cw * (ch + 1)],
                        AF.Exp,
                        bias=bi_t[:, i : i + 1], scale=sc_t[:, i : i + 1],
                    )
                for b in range(F // 512):
                    nc.tensor.matmul(
                        integ[:, 512 * b : 512 * (b + 1)],
                        w_t[:],
                        e[:, 512 * b : 512 * (b + 1)],
                        start=(i == 0),
                        stop=(i == NI - 1),
                    )

            mge_g = singles.tile([G, F], fp32)
            integ_n = singles.tile([P, FN], fp32)
            for c in range(D):
                nc.any.tensor_copy(
                    mge_g[:, FN * c : FN * (c + 1)],
                    integ[:, FN * c : FN * (c + 1)],
                )
                nc.sync.dma_start(
                    integ_n[G * c : G * (c + 1), :],
                    mge_g[:, FN * c : FN * (c + 1)],
                )
            vc2 = singles.tile([P, FN], fp32)
            tv = singles.tile([P, FN], fp32)
            lntv = singles.tile([P, FN], fp32)
            v = singles.tile([P, FN], fp32)
            HF = FN // 2
            for h in range(2):
                s = slice(HF * h, HF * (h + 1))
                nc.vector.tensor_tensor(vc2[:, s], integ_n[:, s], bh_n[:, s], OP.add)
                nc.vector.tensor_tensor(tv[:, s], vc2[:, s], r2[:, s], OP.mult)
                nc.scalar.activation(lntv[:, s], tv[:, s], AF.Ln)
                nc.scalar.activation(
                    v[:, s], lntv[:, s], AF.Exp, bias=ep_t[:, 2:3], scale=0.5
                )
                nc.sync.dma_start(out[:, s], v[:, s])

    nc.compile()
    _BASS_CACHE["general"] = nc
    return nc


def _host_coeffs(surf, sigma, qobs, M_to_L, inc, m_bh):
    surf = surf.astype(np.float64)
    sigma = sigma.astype(np.float64)
    qobs = qobs.astype(np.float64)
    cos_i, sin_i = np.cos(inc), np.sin(inc)
    q_intr = np.sqrt(qobs**2 - cos_i**2) / sin_i
    md = surf * M_to_L * qobs / (q_intr * sigma * np.sqrt(2.0 * np.pi))
    scale = np.quantile(sigma, 0.5)
    sig_sc = sigma / scale
    mds = np.quantile(sig_sc, 0.5)
    mxs = sig_sc.max()
    t_lo = np.arcsinh(np.log(1e-7 * mds) * 2.0 / np.pi)
    t_hi = np.arcsinh(np.log(1000.0 * mxs) * 2.0 / np.pi)
    xl, wl = leggauss(QUAD)
    t = 0.5 * (t_hi - t_lo) * xl + 0.5 * (t_hi + t_lo)
    w = 0.5 * (t_hi - t_lo) * wl
    u = np.exp(np.pi / 2.0 * np.sinh(t))
    du = np.pi / 2.0 * np.cosh(t) * u
    coef = q_intr * md
    inv_s2 = 1.0 / sig_sc**2
    a_j = 0.5 / (1.0 + u)
    b = (a_j[:, None] * inv_s2[None, :]).ravel()
    c = (
        (coef[None, :] / ((1.0 + u[:, None]) ** 2
                          * np.sqrt(q_intr[None, :] ** 2 + u[:, None])))
        * (du * w)[:, None]
    ).ravel()
    b_eff = b / scale**2
    mge_c = 2.0 * np.pi * G_CONST * scale**2
    c = c * mge_c
    bh_bias = np.log(G_CONST) + m_bh * np.log(10.0) + 2.0 * np.log(scale)
    v_bias = -np.log(scale)
    return b_eff, c, mge_c, bh_bias, v_bias


def _run_general(x, y, z, surf, sigma, qobs, M_to_L, inc, m_bh):
    from concourse.bass_utils import run_bass_kernel_spmd

    b_eff, c, mge_c, bh_bias, v_bias = _host_coeffs(
        np.asarray(surf), np.asarray(sigma), np.asarray(qobs),
        float(M_to_L), float(inc), float(m_bh),
    )
    jj = np.arange(P) // G
    scale_sb = np.empty((P, NI), np.float32)
    bias_sb = np.empty((P, NI), np.float32)
    for i in range(NI):
        m = D * i + jj
        scale_sb[:, i] = -b_eff[m]
        bias_sb[:, i] = np.log(c[m])
    w_red = np.zeros((P, G), np.float16)
    w_red[np.arange(P), np.arange(P) % G] = 1.0
    eplg = np.zeros((P, 4), np.float32)
    eplg[:, 0] = bh_bias
    eplg[:, 1] = mge_c
    eplg[:, 2] = v_bias

    xf = x.ravel().reshape(N_CORES, P, FN)
    yf = y.ravel().reshape(N_CORES, P, FN)
    zf = z.ravel().reshape(N_CORES, P, FN)
    in_maps = [
        {
            "xs": xf[i], "ys": yf[i], "zs": zf[i],
            "w_red": w_red, "scale_sb": scale_sb, "bias_sb": bias_sb,
            "eplg": eplg,
        }
        for i in range(N_CORES)
    ]
    nc = _build_general()
    res = run_bass_kernel_spmd(nc, in_maps, core_ids=list(range(N_CORES)))
    outs = [res.results[i]["out"].reshape(-1) for i in range(N_CORES)]
    return np.concatenate(outs).reshape(H, W).astype(np.float32)


def kernel(x, y, z, surf, sigma, qobs, M_to_L, inc, m_bh, quad_points):
    x = np.asarray(x, dtype=np.float32)
    y = np.asarray(y, dtype=np.float32)
    z = np.asarray(z, dtype=np.float32)
    surf = np.asarray(surf)
    sigma = np.asarray(sigma)
    qobs = np.asarray(qobs)

    bound, k = _fast_path_bound(
        x, y, z, surf, sigma, qobs, float(M_to_L), float(inc), float(m_bh)
    )
    if k is not None and bound < 1e-3:
        # pre-scale so fp16 squares stay clear of the denormal zone while
        # not overflowing; verify the residual flush risk on the actual data
        amax = max(np.abs(x).max(), np.abs(y).max(), np.abs(z).max())
        if np.isfinite(amax) and amax > 0:
            B = float(2.0 ** np.floor(np.log2(np.sqrt(60000.0) / amax)))
        else:
            B = 1.0
        if B >= 0.25:
            xh = (x.astype(np.float64) * B).astype(np.float16).astype(np.float64)
            yh = (y.astype(np.float64) * B).astype(np.float16).astype(np.float64)
            zh = (z.astype(np.float64) * B).astype(np.float16).astype(np.float64)
            sq = np.stack([xh * xh, yh * yh, zh * zh])
            r2 = sq.sum(axis=0)
            loss = np.where(sq < 6.103515625e-05, sq, 0.0).sum(axis=0)
            with np.errstate(invalid="ignore", divide="ignore"):
                ftz = float(np.max(loss / np.maximum(r2, 1e-300)))
            if np.isfinite(ftz) and 0.25 * ftz < 5e-3:
                return _run_fast(x, y, z, k, B)
    return _run_general(
        x, y, z, surf, sigma, qobs, float(M_to_L), float(inc), float(m_bh)
    )
